# revision 22
# baseline (speedup 1.0000x reference)
"""EquivariantGraphConvolution (EGNN layer) on 8 Trainium2 NeuronCores.

Strategy (v3.2)
---------------
Nodes are range-partitioned across the 8 cores (6250 nodes each); every edge is
owned by the core that owns its *start* node, so the per-start segment sums are
core-local and no collective is needed.  Per core, edges are bucketed by
128-node start block and padded to a static layout of 49 blocks x NCH chunks x
128 edges, NCH chosen from the data (ceil(max block edges / 128), even).

Key design points (see earlier versions for the v2 gather design):
  * No dma_gather anywhere: the host materializes [nf[start] ; nf[end]] per
    edge slot as one [128, NCH*128] bf16 stream per block (128B/edge).
    Stage 1 is 2 col-tiled matmuls with the [W1a; W1b] stationary plus one
    for the ef/coord-norm terms.
  * The scatter one-hot is built on-chip with one DVE is_equal per block.
  * Segment-sum: rcg [128 edges, 67] bf16 is the matmul stationary, one-hot
    moving, accumulating aggT [67 feats, 128 nodes] over NCH chunks.
  * Activation-table discipline: only Silu/Tanh/Identity (one table set).
    Gate = 0.5*(1+tanh(0.5 logit + 0.5 b_i)).
  * 1/deg is folded into the coord stream, b_n2 into a DVE add; the nf
    output is written feature-major so no transposes are needed in the
    node-update phase; velocity scales go through a DRAM round-trip to
    avoid 49 PE transposes.
"""
import sys
sys.path.insert(0, "/opt/trn_rl_repo")
import contextlib
import os
import numpy as np

import concourse.bass as bass
import concourse.bacc as bacc
import concourse.mybir as mybir
import concourse.tile as tile
from concourse.bass_utils import run_bass_kernel_spmd

f32 = mybir.dt.float32
bf16 = mybir.dt.bfloat16
AF = mybir.ActivationFunctionType
OP = mybir.AluOpType

# ---- problem constants (hardcoded per contract) ----
N = 50000
E = 1_000_000
H = 64
EF = 16
NCORES = 8
NPC = N // NCORES          # 6250 nodes per core
NBLK = 49                  # 128-node blocks per core (49*128 = 6272 >= 6250)
NPAD = NBLK * 128          # 6272
GRP = 6                    # max chunks per flip/gate group

# CoreSim does not implement Silu; substitute Tanh for structural sim checks.
SIM_ACT = os.environ.get("EGNN_SIMACT", "0") == "1"
AF_SILU = AF.Tanh if SIM_ACT else AF.Silu

_cache = {}


def _sup_widths(half):
    """Supertile widths covering `half` columns (each <= 512, mult of 128)."""
    w = [512] * (half // 512)
    if half % 512:
        w.append(half % 512)
    return w


def _f(x):
    return np.ascontiguousarray(x, np.float32)


def _prep_weights(inp):
    """Small weight/constant tensors, identical on all cores."""
    W_e1 = _f(inp["W_e1"])           # [145, 64]
    w = {}
    w["wse"] = W_e1[0:128].copy()                    # [128, 64] start|end rows
    wef = np.zeros((34, 128), np.float32)
    wef[0:16, 0:64] = W_e1[129:145]
    wef[16:17, 0:64] = W_e1[128:129]
    wef[17:33, 64:128] = W_e1[129:145]
    wef[33:34, 64:128] = W_e1[128:129]
    w["wefcdn"] = wef
    W_e2 = _f(inp["W_e2"]); W_c1 = _f(inp["W_c1"])
    bd = np.zeros((128, 128), np.float32)
    bd[0:64, 0:64] = W_e2; bd[64:128, 64:128] = W_e2
    w["wde2"] = bd
    # swapped block-diagonal: cpx rows 0:64 = W_c1^T msgB, 64:128 = W_c1^T msgA
    bcx = np.zeros((128, 128), np.float32)
    bcx[64:128, 0:64] = W_c1
    bcx[0:64, 64:128] = W_c1
    w["wdc1x"] = bcx
    W_i = _f(inp["W_i"]); W_c2 = _f(inp["W_c2"])
    # mcA = [msgA ; chA]  -> out cols: 0:64 msg, 64 gate logit, 65 coord weight
    ra = np.zeros((128, 66), np.float32)
    ra[0:64, 0:64] = np.eye(64); ra[0:64, 64:65] = W_i; ra[64:128, 65:66] = W_c2
    w["rawcA"] = ra
    # mcB = [chB ; msgB]
    rb = np.zeros((128, 66), np.float32)
    rb[64:128, 0:64] = np.eye(64); rb[64:128, 64:65] = W_i; rb[0:64, 65:66] = W_c2
    w["rawcB"] = rb
    w["wn1a"] = _f(inp["W_n1"])[0:64].copy()          # [64, 64] nf rows
    w["wn1b"] = _f(inp["W_n1"])[64:128].copy()        # [64, 64] agg rows
    w["wn2"] = _f(inp["W_n2"])
    w["wv1"] = _f(inp["W_v1"])
    w["wv2"] = _f(inp["W_v2"])
    w["eye3"] = np.eye(3, dtype=np.float32)
    w["arange128"] = np.tile(np.arange(128, dtype=np.float32), (128, 1))

    b_e1 = _f(inp["b_e1"]); b_e2 = _f(inp["b_e2"]); b_c1 = _f(inp["b_c1"])
    w["be1s"] = np.concatenate([b_e1, b_e1]).reshape(128, 1)
    w["be2s"] = np.concatenate([b_e2, b_e2]).reshape(128, 1)
    w["bc1s"] = np.concatenate([b_c1, b_c1]).reshape(128, 1)
    w["bih"] = np.full((128, 1), 0.5 * float(np.asarray(inp["b_i"]).ravel()[0]), np.float32)
    w["bn1c"] = _f(inp["b_n1"]).reshape(64, 1)
    w["bn2c"] = _f(inp["b_n2"]).reshape(64, 1)
    w["bv1c"] = _f(inp["b_v1"]).reshape(64, 1)
    w["bv2c"] = np.full((1, 1), float(np.asarray(inp["b_v2"]).ravel()[0]), np.float32)
    return w


def _prep_core(c, NCH, start, end, ef, nfi, nf_bf, cd_all, cdn_all, invcnt_all):
    bfdt = mybir.dt.np(bf16)
    BLKE = NCH * 128
    ES = NBLK * BLKE
    HALF = BLKE // 2
    lo, hi = c * NPC, (c + 1) * NPC
    sel = (start >= lo) & (start < hi)
    eo = np.nonzero(sel)[0]
    s_loc = (start[eo] - lo).astype(np.int64)
    blk = s_loc >> 7
    order = np.argsort(blk, kind="stable")
    eo = eo[order]; s_loc = s_loc[order]; blk = blk[order]
    counts = np.bincount(blk, minlength=NBLK)
    if counts.max() > BLKE:
        raise RuntimeError(f"block overflow: {counts.max()} > {BLKE}")
    starts_ = np.zeros(NBLK, np.int64)
    starts_[1:] = np.cumsum(counts)[:-1]
    within = np.arange(len(eo)) - starts_[blk]
    slots = blk * BLKE + within

    sg = np.full(ES, lo, np.int64)       # global start per slot (pad -> node lo)
    eg = np.zeros(ES, np.int64)          # global end per slot (pad -> node 0)
    lid = np.full(ES, -1.0, np.float32)
    cds = np.zeros((ES, 3), np.float32)
    cdns = np.zeros(ES, np.float32)
    efs = np.zeros((ES, EF), np.float32)
    sg[slots] = start[eo]
    eg[slots] = end[eo]
    lid[slots] = (s_loc & 127).astype(np.float32)
    cds[slots] = cd_all[eo] * invcnt_all[start[eo]][:, None]
    cdns[slots] = cdn_all[eo]
    efs[slots] = ef[eo]

    d = {}
    # [NBLK, 128, BLKE] bf16: rows 0:64 = nf[start].T, 64:128 = nf[end].T
    nfse = np.empty((ES, 128), bfdt)
    nfse[:, 0:64] = nf_bf[sg]
    nfse[:, 64:128] = nf_bf[eg]
    d["nfse"] = np.ascontiguousarray(
        nfse.reshape(NBLK, BLKE, 128).transpose(0, 2, 1))
    # scatter-orientation lid stream [NBLK, 128, NCH] (edge-in-chunk, chunk)
    lidc = lid.reshape(NBLK, NCH, 128)
    d["lidc"] = np.ascontiguousarray(lidc.transpose(0, 2, 1)).astype(bfdt)
    d["cdem"] = cds.reshape(NBLK, NCH, 128, 3).transpose(0, 2, 1, 3).copy()
    # ef|cdn feature-major: rows 0:17 = A-half edge, 17:34 = B-half edge
    efcdn = np.empty((NBLK, 2, HALF, EF + 1), np.float32)
    efcdn[:, :, :, 0:EF] = efs.reshape(NBLK, 2, HALF, EF)
    efcdn[:, :, :, EF] = cdns.reshape(NBLK, 2, HALF)
    d["efcdn"] = np.ascontiguousarray(
        efcdn.transpose(0, 1, 3, 2).reshape(NBLK, 34, HALF)).astype(bfdt)

    nmc = np.zeros((NPAD, 6), np.float32)
    nmc[0:NPC] = nfi[lo:hi, 0:6]
    d["nodec"] = nmc.reshape(NBLK, 128, 6).transpose(1, 0, 2).reshape(128, NBLK * 6).copy()
    nl = np.zeros((64, NPAD), np.float32)
    nl[:, 0:NPC] = nfi[lo:hi, 6:70].T
    d["nfT_local"] = nl
    return d


def _build_program(NCH):
    STAGE = int(os.environ.get("EGNN_STAGE", "5"))
    BLKE = NCH * 128
    HALF = BLKE // 2
    SUPW = _sup_widths(HALF)        # supertile widths
    SUPO = [sum(SUPW[:i]) for i in range(len(SUPW))]
    # flip/gate groups: chunks [o, o+n) per group
    GRPS = []
    o = 0
    while o < NCH:
        n = min(GRP, NCH - o)
        GRPS.append((o, n))
        o += n

    nc = bacc.Bacc("TRN2", target_bir_lowering=False, debug=False,
                   enable_asserts=False, num_devices=NCORES)

    def din(name, shape, dt=f32):
        return nc.dram_tensor(name, list(shape), dt, kind="ExternalInput").ap()

    nfse_d = din("nfse", [NBLK, 128, BLKE], bf16)
    lidc_d = din("lidc", [NBLK, 128, NCH], bf16)
    cdem_d = din("cdem", [NBLK, 128, NCH, 3])
    efcdn_d = din("efcdn", [NBLK, 34, HALF], bf16)
    nodec_d = din("nodec", [128, NBLK * 6])
    nfT_loc_d = din("nfT_local", [64, NPAD])
    wnames = ["wse", "wefcdn", "wde2", "wdc1x", "rawcA", "rawcB",
              "wn1a", "wn1b", "wn2", "wv1", "wv2", "eye3", "arange128",
              "be1s", "be2s", "bc1s", "bih", "bn1c", "bn2c", "bv1c", "bv2c"]
    wshapes = {"wse": [128, 64], "wefcdn": [34, 128],
               "wde2": [128, 128], "wdc1x": [128, 128],
               "rawcA": [128, 66], "rawcB": [128, 66],
               "wn1a": [64, 64], "wn1b": [64, 64], "wn2": [64, 64],
               "wv1": [64, 64], "wv2": [64, 1], "eye3": [3, 3],
               "arange128": [128, 128],
               "be1s": [128, 1], "be2s": [128, 1], "bc1s": [128, 1],
               "bih": [128, 1], "bn1c": [64, 1], "bn2c": [64, 1],
               "bv1c": [64, 1], "bv2c": [1, 1]}
    wd = {n: din(n, wshapes[n]) for n in wnames}
    outc_d = nc.dram_tensor("outc", [NPAD, 6], f32, kind="ExternalOutput").ap()
    outT_d = nc.dram_tensor("outT", [64, NPAD], f32, kind="ExternalOutput").ap()
    vs_dram = nc.dram_tensor("vs_dram", [NPAD], f32).ap()

    # weights that are matmul operands in the bf16 pipeline
    BF_W = ("wse", "wefcdn", "wde2", "wdc1x", "rawcA", "rawcB", "arange128")

    with tile.TileContext(nc) as tc, contextlib.ExitStack() as ctx:
        wpool = ctx.enter_context(tc.tile_pool(name="w", bufs=1))
        wt = {}
        for n in wnames:
            dt = bf16 if n in BF_W else f32
            t = wpool.tile(wshapes[n], dt, name=f"wt_{n}")
            if dt == f32:
                nc.sync.dma_start(t[:], wd[n][:])
            else:
                tf = wpool.tile(wshapes[n], f32, name=f"wtf_{n}")
                nc.sync.dma_start(tf[:], wd[n][:])
                nc.vector.tensor_copy(t[:], tf[:])
            wt[n] = t
        nodec = wpool.tile([128, NBLK * 6], f32, name="nodec")
        nc.sync.dma_start(nodec[:], nodec_d[:])
        nfT_loc = wpool.tile([64, NPAD], f32, name="nfT_loc")
        nc.sync.dma_start(nfT_loc[:], nfT_loc_d[:])
        vscale = wpool.tile([128, NBLK], f32, name="vscale")
        aggm = wpool.tile([64, NPAD], f32, name="aggm")
        aggc = wpool.tile([3, NPAD], f32, name="aggc")

        # ---------- Phase B: velocity MLP -> vscale [128, NBLK] ----------
        with tc.tile_pool(name="pb", bufs=2) as pb, \
             tc.tile_pool(name="pbp", bufs=2, space="PSUM") as pbp:
            tiles = [(j * 512, 512) for j in range(NPAD // 512)]
            if NPAD % 512:
                tiles.append((NPAD // 512 * 512, NPAD % 512))
            for (o, L) in tiles:
                vps = pbp.tile([64, L], f32, name=f"vps{o}", tag="vps")
                nc.tensor.matmul(vps[:], wt["wv1"][:], nfT_loc[:, o:o + L])
                vh = pb.tile([64, L], f32, name=f"vh{o}", tag="vh")
                nc.scalar.activation(vh[:], vps[:], AF_SILU, bias=wt["bv1c"][:])
                sps = pbp.tile([1, L], f32, name=f"sps{o}", tag="sps")
                nc.tensor.matmul(sps[:], wt["wv2"][:], vh[:])
                vsc = pb.tile([1, L], f32, name=f"vsc{o}", tag="vsc")
                nc.vector.tensor_scalar_add(vsc[:], sps[:], wt["bv2c"][0:1, 0:1])
                nc.sync.dma_start(vs_dram[o:o + L].unsqueeze(0), vsc[:])
            # read back node-block-major: vscale[p, b] = vs_dram[b*128 + p]
            nc.sync.dma_start(vscale[:],
                              vs_dram[:].rearrange("(b p) -> p b", p=128))

        # ---------- Edge sweep (node update fused per block) ----------
        if STAGE >= 2:
            _edge_sweep(nc, tc, STAGE, NCH, SUPW, SUPO, GRPS, wt,
                        nfse_d, lidc_d, cdem_d, efcdn_d, nfT_loc,
                        aggm, aggc, nodec, vscale, outc_d, outT_d)

    nc.compile()
    return nc


def _edge_sweep(nc, tc, STAGE, NCH, SUPW, SUPO, GRPS, wt,
                nfse_d, lidc_d, cdem_d, efcdn_d, nfT_loc,
                aggm, aggc, nodec, vscale, outc_d, outT_d):
    BLKE = NCH * 128
    HALF = BLKE // 2
    NHC = NCH // 2
    NSUP = len(SUPW)

    def mc_of_chunk(mcs, j):
        """Map chunk j to (mc tile, rawc, 128-col slice within supertile)."""
        half, jj = (0, j) if j < NHC else (1, j - NHC)
        col = jj * 128
        for s in range(NSUP):
            if col < SUPO[s] + SUPW[s]:
                off = col - SUPO[s]
                rawc = wt["rawcA"] if half == 0 else wt["rawcB"]
                return mcs[s][half], rawc, slice(off, off + 128)
        raise AssertionError

    with tc.tile_pool(name="pg", bufs=3) as pg, \
         tc.tile_pool(name="pcd", bufs=5) as pcd, \
         tc.tile_pool(name="ph", bufs=3) as ph, \
         tc.tile_pool(name="pe", bufs=7) as pe, \
         tc.tile_pool(name="pch", bufs=6) as pch, \
         tc.tile_pool(name="poh", bufs=2) as poh, \
         tc.tile_pool(name="px1", bufs=3, space="PSUM") as px1, \
         tc.tile_pool(name="pmc", bufs=2, space="PSUM") as pmc, \
         tc.tile_pool(name="pst", bufs=2, space="PSUM") as pst, \
         tc.tile_pool(name="pagg", bufs=1, space="PSUM") as pagg:

        def phase_c(b):
            cols = slice(b * 128, (b + 1) * 128)
            n1 = pagg.tile([64, 128], f32, name=f"n1{b}", tag="aggT")
            nc.tensor.matmul(n1[:], wt["wn1a"][:], nfT_loc[:, cols],
                             start=True, stop=False)
            nc.tensor.matmul(n1[:], wt["wn1b"][:], aggm[:, cols],
                             start=False, stop=True)
            hn = pch.tile([64, 128], f32, name=f"hn{b}", tag="hn")
            nc.scalar.activation(hn[:], n1[:], AF_SILU, bias=wt["bn1c"][:])
            n2 = pagg.tile([64, 128], f32, name=f"n2{b}", tag="aggT")
            nc.tensor.matmul(n2[:], wt["wn2"][:], hn[:])
            hn2 = pch.tile([64, 128], f32, name=f"hn2{b}", tag="hn2")
            nc.vector.tensor_tensor(
                hn2[:], n2[:], wt["bn2c"][:].broadcast_to([64, 128]), OP.add)
            ot67 = pch.tile([64, 128], f32, name=f"ot67{b}", tag="ot67")
            nc.vector.tensor_tensor(ot67[:], nfT_loc[:, cols], hn2[:], OP.add)
            nc.sync.dma_start(outT_d[:, cols], ot67[:])
            # coords/vels (node-major)
            nmb = nodec[:, b * 6:(b + 1) * 6]
            ctp = pagg.tile([128, 3], f32, name=f"ctp{b}", tag="aggT")
            nc.tensor.transpose(ctp[:], aggc[:, cols], wt["eye3"][:])
            otc = pch.tile([128, 6], f32, name=f"otc{b}", tag="otc")
            t2 = pch.tile([128, 3], f32, name=f"t2{b}", tag="t2")
            nc.vector.tensor_tensor(
                t2[:], nmb[:, 3:6],
                vscale[:, b:b + 1].broadcast_to([128, 3]), OP.mult)
            nc.vector.tensor_copy(otc[:, 3:6], nmb[:, 3:6])
            t3 = pch.tile([128, 3], f32, name=f"t3{b}", tag="t3")
            nc.vector.tensor_tensor(t3[:], ctp[:], t2[:], OP.add)
            nc.vector.tensor_tensor(otc[:, 0:3], t3[:], nmb[:, 0:3], OP.add)
            nc.sync.dma_start(outc_d[b * 128:(b + 1) * 128, :], otc[:])

        state = {}

        def front_dma(b):
            nfse = pg.tile([128, BLKE], bf16, name=f"nfse{b}", tag="nfse")
            nc.sync.dma_start(nfse[:], nfse_d[b])
            lidt = pg.tile([128, NCH], bf16, name=f"lidt{b}", tag="lidt")
            nc.sync.dma_start(lidt[:], lidc_d[b])
            cdl = pcd.tile([128, NCH, 3], f32, name=f"cdl{b}", tag="cdl")
            nc.sync.dma_start(cdl[:], cdem_d[b])
            eftb = pg.tile([34, HALF], bf16, name=f"eftb{b}", tag="eftb")
            nc.sync.dma_start(eftb[:], efcdn_d[b])

            # scatter one-hot [128 edge-in-chunk, NCH chunk, 128 node] on-chip
            ohts = poh.tile([128, NCH, 128], bf16, name=f"ohts{b}", tag="ohts")
            nc.vector.tensor_tensor(
                ohts[:],
                lidt[:].unsqueeze(2).broadcast_to([128, NCH, 128]),
                wt["arange128"][:].unsqueeze(1).broadcast_to([128, NCH, 128]),
                OP.is_equal)
            state[b] = {"nfse": nfse, "cdl": cdl, "eftb": eftb, "ohts": ohts,
                        "h1s": [], "mcs": []}
            if STAGE == 2:
                nc.any.tensor_copy(aggm[:, b * 128:(b + 1) * 128],
                                   ohts[0:64, 0, :])

        def front_s1(b, s):
            stt = state[b]
            w = SUPW[s]
            sl = slice(SUPO[s], SUPO[s] + w)
            slh = slice(HALF + SUPO[s], HALF + SUPO[s] + w)
            x1 = px1.tile([128, w], f32, name=f"x1{b}_{s}", tag="x1")
            nc.tensor.matmul(x1[0:64, :], wt["wse"][:], stt["nfse"][:, sl],
                             start=True, stop=False, skip_group_check=True)
            nc.tensor.matmul(x1[64:128, :], wt["wse"][:], stt["nfse"][:, slh],
                             start=True, stop=False,
                             tile_position=(0, 64), skip_group_check=True)
            nc.tensor.matmul(x1[:], wt["wefcdn"][:], stt["eftb"][:, sl],
                             start=False, stop=True, skip_group_check=True)
            h1 = ph.tile([128, w], bf16, name=f"h1{b}_{s}", tag="h1")
            nc.scalar.activation(h1[:], x1[:], AF_SILU, bias=wt["be1s"][:])
            stt["h1s"].append(h1)

        MC_GPS = True

        def front_s23(b, s):
            stt = state[b]
            w = SUPW[s]
            mp = pmc.tile([128, w], f32, name=f"mp{b}_{s}", tag="mmid")
            nc.tensor.matmul(mp[:], wt["wde2"][:], stt["h1s"][s][:])
            mcA = pe.tile([128, w], bf16, name=f"mcA{b}_{s}", tag="mcA")
            mcB = pe.tile([128, w], bf16, name=f"mcB{b}_{s}", tag="mcB")
            if MC_GPS:
                # full-lane silus (ACT time scales with free dim only); the
                # lane-aligned mc assembly copies run on the idle GpSimd
                msgAB = pe.tile([128, w], bf16, name=f"msgAB{b}_{s}", tag="msgAB")
                nc.scalar.activation(msgAB[:], mp[:], AF_SILU, bias=wt["be2s"][:])
                nc.gpsimd.tensor_copy(mcA[0:64, :], msgAB[0:64, :])
                nc.gpsimd.tensor_copy(mcB[64:128, :], msgAB[64:128, :])
                cpx = pmc.tile([128, w], f32, name=f"cp{b}_{s}", tag="mmid")
                nc.tensor.matmul(cpx[:], wt["wdc1x"][:], msgAB[:])
                chSW = pe.tile([128, w], bf16, name=f"chSW{b}_{s}", tag="chSW")
                nc.scalar.activation(chSW[:], cpx[:], AF_SILU, bias=wt["bc1s"][:])
                nc.gpsimd.tensor_copy(mcA[64:128, :], chSW[64:128, :])
                nc.gpsimd.tensor_copy(mcB[0:64, :], chSW[0:64, :])
            else:
                raise RuntimeError("only EGNN_MC=gps supported")
            stt["mcs"].append((mcA, mcB))
            if STAGE == 3 and s == 0:
                nc.any.tensor_copy(aggm[:, b * 128:(b + 1) * 128],
                                   mcA[0:64, 0:128])

        def flip_group(b, gi):
            stt = state[b]
            if gi == 0:
                stt["rcg"] = pch.tile([128, NCH, 67], bf16,
                                      name=f"rcg{b}", tag="rcg")
            rcg, cdl = stt["rcg"], stt["cdl"]
            (go, gn) = GRPS[gi]
            st = pst.tile([128, gn, 66], f32, name=f"st{b}_{go}", tag="st")
            for cg in range(gn):
                mc, rawc, cc = mc_of_chunk(stt["mcs"], go + cg)
                nc.tensor.matmul(st[:, cg, :], mc[:, cc], rawc[:],
                                 start=True, stop=True)
            jb = slice(go, go + gn)
            tnh = pch.tile([128, gn], f32, name=f"tnh{b}_{go}", tag="tnh")
            nc.scalar.activation(tnh[:], st[:, :, 64:65].squeeze(2),
                                 AF.Tanh, bias=wt["bih"][:], scale=0.5)
            gate = pch.tile([128, gn], f32, name=f"gt{b}_{go}", tag="gate")
            nc.vector.tensor_scalar(out=gate[:], in0=tnh[:], scalar1=1.0,
                                    scalar2=0.5, op0=OP.add, op1=OP.mult)
            nc.vector.tensor_tensor(
                rcg[:, jb, 0:64], st[:, :, 0:64],
                gate[:].unsqueeze(2).broadcast_to([128, gn, 64]), OP.mult)
            nc.vector.tensor_tensor(
                rcg[:, jb, 64:67], cdl[:, jb, :],
                st[:, :, 65:66].broadcast_to([128, gn, 3]), OP.mult)

        def scatter_part(b, j0, j1):
            stt = state[b]
            if j0 == 0:
                stt["aggT"] = pagg.tile([67, 128], f32,
                                        name=f"aggT{b}", tag="aggT")
            aggT, rcg, ohts = stt["aggT"], stt["rcg"], stt["ohts"]
            for j in range(j0, j1):
                nc.tensor.matmul(aggT[:, :], rcg[:, j, :],
                                 ohts[:, j, :],
                                 start=(j == 0), stop=(j == NCH - 1))
            if j1 == NCH:
                nc.vector.tensor_copy(aggm[:, b * 128:(b + 1) * 128],
                                      aggT[0:64, :])
                nc.vector.tensor_copy(aggc[:, b * 128:(b + 1) * 128],
                                      aggT[64:67, :])
                state.pop(b)

        def back_steps(b):
            steps = [lambda gi=gi: flip_group(b, gi) for gi in range(len(GRPS))]
            steps.append(lambda: scatter_part(b, 0, NCH // 2))
            steps.append(lambda: scatter_part(b, NCH // 2, NCH))
            return steps

        def front_steps(b):
            steps = [lambda s=s: front_s1(b, s) for s in range(NSUP)]
            if STAGE >= 3:
                steps += [lambda s=s: front_s23(b, s) for s in range(NSUP)]
            return steps

        def front(b):
            front_dma(b)
            if STAGE >= 3:
                for f in front_steps(b):
                    f()

        def back(b):
            for s in back_steps(b):
                s()

        for b in range(NBLK):
            front(b)
            if STAGE >= 4 and b > 0:
                back(b - 1)
            if STAGE >= 5 and b > 1:
                phase_c(b - 2)
        if STAGE >= 4:
            back(NBLK - 1)
        if STAGE >= 5:
            phase_c(NBLK - 2)
            phase_c(NBLK - 1)


def kernel(**inputs):
    ei = np.asarray(inputs["edge_indices"])
    start = ei[0].astype(np.int64)
    end = ei[1].astype(np.int64)
    ef = _f(inputs["edge_features"])
    nfi = _f(inputs["node_features_input"])
    coords = nfi[:, 0:3]
    cd_all = coords[start] - coords[end]
    cdn_all = np.sqrt((cd_all ** 2).sum(1)).astype(np.float32)
    deg = np.bincount(start, minlength=N).astype(np.float32)
    invcnt_all = (1.0 / np.maximum(deg, 1.0)).astype(np.float32)
    nf_bf = nfi[:, 6:70].astype(mybir.dt.np(bf16))

    # chunk count from data: ceil(max block load / 128), rounded up to even
    mx = 0
    for c in range(NCORES):
        lo, hi = c * NPC, (c + 1) * NPC
        s = start[(start >= lo) & (start < hi)] - lo
        cnt = np.bincount(s >> 7, minlength=NBLK)
        mx = max(mx, int(cnt.max()))
    NCH = -(-mx // 128)
    NCH += NCH % 2
    NCH = max(NCH, 8)

    w = _prep_weights(inputs)

    in_maps = []
    for c in range(NCORES):
        d = _prep_core(c, NCH, start, end, ef, nfi, nf_bf, cd_all, cdn_all,
                       invcnt_all)
        d.update(w)
        in_maps.append(d)

    if _cache.get("NCH") != NCH:
        _cache["NCH"] = NCH
        _cache["nc"] = _build_program(NCH)
    nc = _cache["nc"]
    _cache["in_maps"] = in_maps
    res = run_bass_kernel_spmd(nc, in_maps, list(range(NCORES)))
    out = np.empty((N, 70), np.float32)
    for c in range(NCORES):
        out[c * NPC:(c + 1) * NPC, 0:6] = res.results[c]["outc"][0:NPC]
        out[c * NPC:(c + 1) * NPC, 6:70] = res.results[c]["outT"][:, 0:NPC].T
    return out


# revision 23
# speedup vs baseline: 1.7264x; 1.7264x over previous
"""EquivariantGraphConvolution (EGNN layer) on 8 Trainium2 NeuronCores.

Strategy (v3.2)
---------------
Nodes are range-partitioned across the 8 cores (6250 nodes each); every edge is
owned by the core that owns its *start* node, so the per-start segment sums are
core-local and no collective is needed.  Per core, edges are bucketed by
128-node start block and padded to a static layout of 49 blocks x NCH chunks x
128 edges, NCH chosen from the data (ceil(max block edges / 128), even).

Key design points (see earlier versions for the v2 gather design):
  * No dma_gather anywhere: the host materializes [nf[start] ; nf[end]] per
    edge slot as one [128, NCH*128] bf16 stream per block (128B/edge).
    Stage 1 is 2 col-tiled matmuls with the [W1a; W1b] stationary plus one
    for the ef/coord-norm terms.
  * The scatter one-hot is built on-chip with one DVE is_equal per block.
  * Segment-sum: rcg [128 edges, 67] bf16 is the matmul stationary, one-hot
    moving, accumulating aggT [67 feats, 128 nodes] over NCH chunks.
  * Activation-table discipline: only Silu/Tanh/Identity (one table set).
    Gate = 0.5*(1+tanh(0.5 logit + 0.5 b_i)).
  * 1/deg is folded into the coord stream, b_n2 into a DVE add; the nf
    output is written feature-major so no transposes are needed in the
    node-update phase; velocity scales go through a DRAM round-trip to
    avoid 49 PE transposes.
"""
import sys
sys.path.insert(0, "/opt/trn_rl_repo")
import contextlib
import os
import numpy as np

import concourse.bass as bass
import concourse.bacc as bacc
import concourse.mybir as mybir
import concourse.tile as tile
from concourse.bass_utils import run_bass_kernel_spmd

f32 = mybir.dt.float32
bf16 = mybir.dt.bfloat16
AF = mybir.ActivationFunctionType
OP = mybir.AluOpType

# ---- problem constants (hardcoded per contract) ----
N = 50000
E = 1_000_000
H = 64
EF = 16
NCORES = 8
NPC = N // NCORES          # 6250 nodes per core
NBLK = 49                  # 128-node blocks per core (49*128 = 6272 >= 6250)
NPAD = NBLK * 128          # 6272
GRP = 6                    # max chunks per flip/gate group

# CoreSim does not implement Silu; substitute Tanh for structural sim checks.
SIM_ACT = os.environ.get("EGNN_SIMACT", "0") == "1"
AF_SILU = AF.Tanh if SIM_ACT else AF.Silu

_cache = {}


def _sup_widths(half):
    """Supertile widths covering `half` columns (each <= 512, mult of 128)."""
    w = [512] * (half // 512)
    if half % 512:
        w.append(half % 512)
    return w


def _f(x):
    return np.ascontiguousarray(x, np.float32)


def _prep_weights(inp):
    """Small weight/constant tensors, identical on all cores."""
    W_e1 = _f(inp["W_e1"])           # [145, 64]
    w = {}
    w["wse"] = W_e1[0:128].copy()                    # [128, 64] start|end rows
    wef = np.zeros((34, 128), np.float32)
    wef[0:16, 0:64] = W_e1[129:145]
    wef[16:17, 0:64] = W_e1[128:129]
    wef[17:33, 64:128] = W_e1[129:145]
    wef[33:34, 64:128] = W_e1[128:129]
    w["wefcdn"] = wef
    W_e2 = _f(inp["W_e2"]); W_c1 = _f(inp["W_c1"])
    bd = np.zeros((128, 128), np.float32)
    bd[0:64, 0:64] = W_e2; bd[64:128, 64:128] = W_e2
    w["wde2"] = bd
    w["wc1d"] = np.concatenate([W_c1, W_c1], axis=0)   # [128, 64], both halves
    W_i = _f(inp["W_i"]); W_c2 = _f(inp["W_c2"])
    # mcA = [msgA ; chA]  -> out cols: 0:64 msg, 64 gate logit, 65 coord weight
    ra = np.zeros((128, 66), np.float32)
    ra[0:64, 0:64] = np.eye(64); ra[0:64, 64:65] = W_i; ra[64:128, 65:66] = W_c2
    w["rawcA"] = ra
    # mcB = [chB ; msgB]
    rb = np.zeros((128, 66), np.float32)
    rb[64:128, 0:64] = np.eye(64); rb[64:128, 64:65] = W_i; rb[0:64, 65:66] = W_c2
    w["rawcB"] = rb
    w["wn1a"] = _f(inp["W_n1"])[0:64].copy()          # [64, 64] nf rows
    w["wn1b"] = _f(inp["W_n1"])[64:128].copy()        # [64, 64] agg rows
    w["wn2"] = _f(inp["W_n2"])
    w["wv1"] = _f(inp["W_v1"])
    w["wv2"] = _f(inp["W_v2"])
    w["eye3"] = np.eye(3, dtype=np.float32)
    w["arange128"] = np.tile(np.arange(128, dtype=np.float32), (128, 1))

    b_e1 = _f(inp["b_e1"]); b_e2 = _f(inp["b_e2"]); b_c1 = _f(inp["b_c1"])
    w["be1s"] = np.concatenate([b_e1, b_e1]).reshape(128, 1)
    w["be2s"] = np.concatenate([b_e2, b_e2]).reshape(128, 1)
    w["bc1s"] = np.concatenate([b_c1, b_c1]).reshape(128, 1)
    w["bih"] = np.full((128, 1), 0.5 * float(np.asarray(inp["b_i"]).ravel()[0]), np.float32)
    w["bn1c"] = _f(inp["b_n1"]).reshape(64, 1)
    w["bn2c"] = _f(inp["b_n2"]).reshape(64, 1)
    w["bv1c"] = _f(inp["b_v1"]).reshape(64, 1)
    w["bv2c"] = np.full((1, 1), float(np.asarray(inp["b_v2"]).ravel()[0]), np.float32)
    return w


def _prep_core(c, NCH, start, end, ef, nfi, nf_bf, cd_all, cdn_all, invcnt_all):
    bfdt = mybir.dt.np(bf16)
    BLKE = NCH * 128
    ES = NBLK * BLKE
    HALF = BLKE // 2
    lo, hi = c * NPC, (c + 1) * NPC
    sel = (start >= lo) & (start < hi)
    eo = np.nonzero(sel)[0]
    s_loc = (start[eo] - lo).astype(np.int64)
    blk = s_loc >> 7
    order = np.argsort(blk, kind="stable")
    eo = eo[order]; s_loc = s_loc[order]; blk = blk[order]
    counts = np.bincount(blk, minlength=NBLK)
    if counts.max() > BLKE:
        raise RuntimeError(f"block overflow: {counts.max()} > {BLKE}")
    starts_ = np.zeros(NBLK, np.int64)
    starts_[1:] = np.cumsum(counts)[:-1]
    within = np.arange(len(eo)) - starts_[blk]
    slots = blk * BLKE + within

    sg = np.full(ES, lo, np.int64)       # global start per slot (pad -> node lo)
    eg = np.zeros(ES, np.int64)          # global end per slot (pad -> node 0)
    lid = np.full(ES, -1.0, np.float32)
    cds = np.zeros((ES, 3), np.float32)
    cdns = np.zeros(ES, np.float32)
    efs = np.zeros((ES, EF), np.float32)
    sg[slots] = start[eo]
    eg[slots] = end[eo]
    lid[slots] = (s_loc & 127).astype(np.float32)
    cds[slots] = cd_all[eo] * invcnt_all[start[eo]][:, None]
    cdns[slots] = cdn_all[eo]
    efs[slots] = ef[eo]

    d = {}
    # [NBLK, 128, BLKE] bf16: rows 0:64 = nf[start].T, 64:128 = nf[end].T
    nfse = np.empty((ES, 128), bfdt)
    nfse[:, 0:64] = nf_bf[sg]
    nfse[:, 64:128] = nf_bf[eg]
    d["nfse"] = np.ascontiguousarray(
        nfse.reshape(NBLK, BLKE, 128).transpose(0, 2, 1))
    # scatter-orientation lid stream [NBLK, 128, NCH] (edge-in-chunk, chunk)
    lidc = lid.reshape(NBLK, NCH, 128)
    d["lidc"] = np.ascontiguousarray(lidc.transpose(0, 2, 1)).astype(bfdt)
    d["cdem"] = cds.reshape(NBLK, NCH, 128, 3).transpose(0, 2, 1, 3).copy()
    # ef|cdn feature-major: rows 0:17 = A-half edge, 17:34 = B-half edge
    efcdn = np.empty((NBLK, 2, HALF, EF + 1), np.float32)
    efcdn[:, :, :, 0:EF] = efs.reshape(NBLK, 2, HALF, EF)
    efcdn[:, :, :, EF] = cdns.reshape(NBLK, 2, HALF)
    d["efcdn"] = np.ascontiguousarray(
        efcdn.transpose(0, 1, 3, 2).reshape(NBLK, 34, HALF)).astype(bfdt)

    nmc = np.zeros((NPAD, 6), np.float32)
    nmc[0:NPC] = nfi[lo:hi, 0:6]
    d["nodec"] = nmc.reshape(NBLK, 128, 6).transpose(1, 0, 2).reshape(128, NBLK * 6).copy()
    nl = np.zeros((64, NPAD), np.float32)
    nl[:, 0:NPC] = nfi[lo:hi, 6:70].T
    d["nfT_local"] = nl
    return d


def _build_program(NCH):
    STAGE = int(os.environ.get("EGNN_STAGE", "5"))
    BLKE = NCH * 128
    HALF = BLKE // 2
    SUPW = _sup_widths(HALF)        # supertile widths
    SUPO = [sum(SUPW[:i]) for i in range(len(SUPW))]
    # flip/gate groups: chunks [o, o+n) per group
    GRPS = []
    o = 0
    while o < NCH:
        n = min(GRP, NCH - o)
        GRPS.append((o, n))
        o += n

    nc = bacc.Bacc("TRN2", target_bir_lowering=False, debug=False,
                   enable_asserts=False, num_devices=NCORES)

    def din(name, shape, dt=f32):
        return nc.dram_tensor(name, list(shape), dt, kind="ExternalInput").ap()

    nfse_d = din("nfse", [NBLK, 128, BLKE], bf16)
    lidc_d = din("lidc", [NBLK, 128, NCH], bf16)
    cdem_d = din("cdem", [NBLK, 128, NCH, 3])
    efcdn_d = din("efcdn", [NBLK, 34, HALF], bf16)
    nodec_d = din("nodec", [128, NBLK * 6])
    nfT_loc_d = din("nfT_local", [64, NPAD])
    wnames = ["wse", "wefcdn", "wde2", "wc1d", "rawcA", "rawcB",
              "wn1a", "wn1b", "wn2", "wv1", "wv2", "eye3", "arange128",
              "be1s", "be2s", "bc1s", "bih", "bn1c", "bn2c", "bv1c", "bv2c"]
    wshapes = {"wse": [128, 64], "wefcdn": [34, 128],
               "wde2": [128, 128], "wc1d": [128, 64],
               "rawcA": [128, 66], "rawcB": [128, 66],
               "wn1a": [64, 64], "wn1b": [64, 64], "wn2": [64, 64],
               "wv1": [64, 64], "wv2": [64, 1], "eye3": [3, 3],
               "arange128": [128, 128],
               "be1s": [128, 1], "be2s": [128, 1], "bc1s": [128, 1],
               "bih": [128, 1], "bn1c": [64, 1], "bn2c": [64, 1],
               "bv1c": [64, 1], "bv2c": [1, 1]}
    wd = {n: din(n, wshapes[n]) for n in wnames}
    outc_d = nc.dram_tensor("outc", [NPAD, 6], f32, kind="ExternalOutput").ap()
    outT_d = nc.dram_tensor("outT", [64, NPAD], f32, kind="ExternalOutput").ap()
    vs_dram = nc.dram_tensor("vs_dram", [NPAD], f32).ap()

    # weights that are matmul operands in the bf16 pipeline
    BF_W = ("wse", "wefcdn", "wde2", "wc1d", "rawcA", "rawcB", "arange128")

    with tile.TileContext(nc) as tc, contextlib.ExitStack() as ctx:
        wpool = ctx.enter_context(tc.tile_pool(name="w", bufs=1))
        wt = {}
        for n in wnames:
            dt = bf16 if n in BF_W else f32
            t = wpool.tile(wshapes[n], dt, name=f"wt_{n}")
            if dt == f32:
                nc.sync.dma_start(t[:], wd[n][:])
            else:
                tf = wpool.tile(wshapes[n], f32, name=f"wtf_{n}")
                nc.sync.dma_start(tf[:], wd[n][:])
                nc.vector.tensor_copy(t[:], tf[:])
            wt[n] = t
        nodec = wpool.tile([128, NBLK * 6], f32, name="nodec")
        nc.sync.dma_start(nodec[:], nodec_d[:])
        nfT_loc = wpool.tile([64, NPAD], f32, name="nfT_loc")
        nc.sync.dma_start(nfT_loc[:], nfT_loc_d[:])
        vscale = wpool.tile([128, NBLK], f32, name="vscale")
        aggm = wpool.tile([64, NPAD], f32, name="aggm")
        aggc = wpool.tile([3, NPAD], f32, name="aggc")

        # ---------- Phase B: velocity MLP -> vscale [128, NBLK] ----------
        with tc.tile_pool(name="pb", bufs=2) as pb, \
             tc.tile_pool(name="pbp", bufs=2, space="PSUM") as pbp:
            tiles = [(j * 512, 512) for j in range(NPAD // 512)]
            if NPAD % 512:
                tiles.append((NPAD // 512 * 512, NPAD % 512))
            for (o, L) in tiles:
                vps = pbp.tile([64, L], f32, name=f"vps{o}", tag="vps")
                nc.tensor.matmul(vps[:], wt["wv1"][:], nfT_loc[:, o:o + L])
                vh = pb.tile([64, L], f32, name=f"vh{o}", tag="vh")
                nc.scalar.activation(vh[:], vps[:], AF_SILU, bias=wt["bv1c"][:])
                sps = pbp.tile([1, L], f32, name=f"sps{o}", tag="sps")
                nc.tensor.matmul(sps[:], wt["wv2"][:], vh[:])
                vsc = pb.tile([1, L], f32, name=f"vsc{o}", tag="vsc")
                nc.vector.tensor_scalar_add(vsc[:], sps[:], wt["bv2c"][0:1, 0:1])
                nc.sync.dma_start(vs_dram[o:o + L].unsqueeze(0), vsc[:])
            # read back node-block-major: vscale[p, b] = vs_dram[b*128 + p]
            nc.sync.dma_start(vscale[:],
                              vs_dram[:].rearrange("(b p) -> p b", p=128))

        # ---------- Edge sweep (node update fused per block) ----------
        if STAGE >= 2:
            _edge_sweep(nc, tc, STAGE, NCH, SUPW, SUPO, GRPS, wt,
                        nfse_d, lidc_d, cdem_d, efcdn_d, nfT_loc,
                        aggm, aggc, nodec, vscale, outc_d, outT_d)

    nc.compile()
    return nc


def _edge_sweep(nc, tc, STAGE, NCH, SUPW, SUPO, GRPS, wt,
                nfse_d, lidc_d, cdem_d, efcdn_d, nfT_loc,
                aggm, aggc, nodec, vscale, outc_d, outT_d):
    BLKE = NCH * 128
    HALF = BLKE // 2
    NHC = NCH // 2
    NSUP = len(SUPW)

    def mc_of_chunk(mcs, j):
        """Map chunk j to (mc tile, rawc, 128-col slice within supertile)."""
        half, jj = (0, j) if j < NHC else (1, j - NHC)
        col = jj * 128
        for s in range(NSUP):
            if col < SUPO[s] + SUPW[s]:
                off = col - SUPO[s]
                rawc = wt["rawcA"] if half == 0 else wt["rawcB"]
                return mcs[s][half], rawc, slice(off, off + 128)
        raise AssertionError

    with tc.tile_pool(name="pg", bufs=3) as pg, \
         tc.tile_pool(name="pcd", bufs=5) as pcd, \
         tc.tile_pool(name="ph", bufs=3) as ph, \
         tc.tile_pool(name="pe", bufs=7) as pe, \
         tc.tile_pool(name="pch", bufs=6) as pch, \
         tc.tile_pool(name="poh", bufs=2) as poh, \
         tc.tile_pool(name="px1", bufs=3, space="PSUM") as px1, \
         tc.tile_pool(name="pmc", bufs=2, space="PSUM") as pmc, \
         tc.tile_pool(name="pst", bufs=2, space="PSUM") as pst, \
         tc.tile_pool(name="pagg", bufs=1, space="PSUM") as pagg:

        def phase_c(b):
            cols = slice(b * 128, (b + 1) * 128)
            n1 = pagg.tile([64, 128], f32, name=f"n1{b}", tag="aggT")
            nc.tensor.matmul(n1[:], wt["wn1a"][:], nfT_loc[:, cols],
                             start=True, stop=False)
            nc.tensor.matmul(n1[:], wt["wn1b"][:], aggm[:, cols],
                             start=False, stop=True)
            hn = pch.tile([64, 128], f32, name=f"hn{b}", tag="hn")
            nc.scalar.activation(hn[:], n1[:], AF_SILU, bias=wt["bn1c"][:])
            n2 = pagg.tile([64, 128], f32, name=f"n2{b}", tag="aggT")
            nc.tensor.matmul(n2[:], wt["wn2"][:], hn[:])
            hn2 = pch.tile([64, 128], f32, name=f"hn2{b}", tag="hn2")
            nc.vector.tensor_tensor(
                hn2[:], n2[:], wt["bn2c"][:].broadcast_to([64, 128]), OP.add)
            ot67 = pch.tile([64, 128], f32, name=f"ot67{b}", tag="ot67")
            nc.vector.tensor_tensor(ot67[:], nfT_loc[:, cols], hn2[:], OP.add)
            nc.sync.dma_start(outT_d[:, cols], ot67[:])
            # coords/vels (node-major)
            nmb = nodec[:, b * 6:(b + 1) * 6]
            ctp = pagg.tile([128, 3], f32, name=f"ctp{b}", tag="aggT")
            nc.tensor.transpose(ctp[:], aggc[:, cols], wt["eye3"][:])
            otc = pch.tile([128, 6], f32, name=f"otc{b}", tag="otc")
            t2 = pch.tile([128, 3], f32, name=f"t2{b}", tag="t2")
            nc.vector.tensor_tensor(
                t2[:], nmb[:, 3:6],
                vscale[:, b:b + 1].broadcast_to([128, 3]), OP.mult)
            nc.vector.tensor_copy(otc[:, 3:6], nmb[:, 3:6])
            t3 = pch.tile([128, 3], f32, name=f"t3{b}", tag="t3")
            nc.vector.tensor_tensor(t3[:], ctp[:], t2[:], OP.add)
            nc.vector.tensor_tensor(otc[:, 0:3], t3[:], nmb[:, 0:3], OP.add)
            nc.sync.dma_start(outc_d[b * 128:(b + 1) * 128, :], otc[:])

        state = {}

        def front_dma(b):
            nfse = pg.tile([128, BLKE], bf16, name=f"nfse{b}", tag="nfse")
            nc.sync.dma_start(nfse[:], nfse_d[b])
            lidt = pg.tile([128, NCH], bf16, name=f"lidt{b}", tag="lidt")
            nc.sync.dma_start(lidt[:], lidc_d[b])
            cdl = pcd.tile([128, NCH, 3], f32, name=f"cdl{b}", tag="cdl")
            nc.sync.dma_start(cdl[:], cdem_d[b])
            eftb = pg.tile([34, HALF], bf16, name=f"eftb{b}", tag="eftb")
            nc.sync.dma_start(eftb[:], efcdn_d[b])

            # scatter one-hot [128 edge-in-chunk, NCH chunk, 128 node] on-chip
            ohts = poh.tile([128, NCH, 128], bf16, name=f"ohts{b}", tag="ohts")
            nc.vector.tensor_tensor(
                ohts[:],
                lidt[:].unsqueeze(2).broadcast_to([128, NCH, 128]),
                wt["arange128"][:].unsqueeze(1).broadcast_to([128, NCH, 128]),
                OP.is_equal)
            state[b] = {"nfse": nfse, "cdl": cdl, "eftb": eftb, "ohts": ohts,
                        "h1s": [], "mcs": []}
            if STAGE == 2:
                nc.any.tensor_copy(aggm[:, b * 128:(b + 1) * 128],
                                   ohts[0:64, 0, :])

        def front_s1(b, s):
            stt = state[b]
            w = SUPW[s]
            sl = slice(SUPO[s], SUPO[s] + w)
            slh = slice(HALF + SUPO[s], HALF + SUPO[s] + w)
            x1 = px1.tile([128, w], f32, name=f"x1{b}_{s}", tag="x1")
            nc.tensor.matmul(x1[0:64, :], wt["wse"][:], stt["nfse"][:, sl],
                             start=True, stop=False, skip_group_check=True)
            nc.tensor.matmul(x1[64:128, :], wt["wse"][:], stt["nfse"][:, slh],
                             start=True, stop=False,
                             tile_position=(0, 64), skip_group_check=True)
            nc.tensor.matmul(x1[:], wt["wefcdn"][:], stt["eftb"][:, sl],
                             start=False, stop=True, skip_group_check=True)
            h1 = ph.tile([128, w], bf16, name=f"h1{b}_{s}", tag="h1")
            nc.scalar.activation(h1[:], x1[:], AF_SILU, bias=wt["be1s"][:])
            stt["h1s"].append(h1)

        def front_s23(b, s):
            stt = state[b]
            w = SUPW[s]
            mp = pmc.tile([128, w], f32, name=f"mp{b}_{s}", tag="mmid")
            nc.tensor.matmul(mp[:], wt["wde2"][:], stt["h1s"][s][:])
            mcA = pe.tile([128, w], bf16, name=f"mcA{b}_{s}", tag="mcA")
            mcB = pe.tile([128, w], bf16, name=f"mcB{b}_{s}", tag="mcB")
            nc.scalar.activation(mcA[0:64, :], mp[0:64, :], AF_SILU,
                                 bias=wt["be2s"][0:64, :])
            nc.scalar.activation(mcB[64:128, :], mp[64:128, :], AF_SILU,
                                 bias=wt["be2s"][64:128, :])
            cpx = pmc.tile([128, w], f32, name=f"cp{b}_{s}", tag="mmid")
            nc.tensor.matmul(cpx[64:128, :], wt["wc1d"][0:64, :], mcA[0:64, :],
                             start=True, stop=True,
                             tile_position=(0, 64), skip_group_check=True)
            nc.tensor.matmul(cpx[0:64, :], wt["wc1d"][64:128, :], mcB[64:128, :],
                             start=True, stop=True,
                             tile_position=(64, 0), skip_group_check=True)
            nc.scalar.activation(mcA[64:128, :], cpx[64:128, :], AF_SILU,
                                 bias=wt["bc1s"][64:128, :])
            nc.scalar.activation(mcB[0:64, :], cpx[0:64, :], AF_SILU,
                                 bias=wt["bc1s"][0:64, :])
            stt["mcs"].append((mcA, mcB))
            if STAGE == 3 and s == 0:
                nc.any.tensor_copy(aggm[:, b * 128:(b + 1) * 128],
                                   mcA[0:64, 0:128])

        def flip_group(b, gi):
            stt = state[b]
            if gi == 0:
                stt["rcg"] = pch.tile([128, NCH, 67], bf16,
                                      name=f"rcg{b}", tag="rcg")
            rcg, cdl = stt["rcg"], stt["cdl"]
            (go, gn) = GRPS[gi]
            st = pst.tile([128, gn, 66], f32, name=f"st{b}_{go}", tag="st")
            for cg in range(gn):
                mc, rawc, cc = mc_of_chunk(stt["mcs"], go + cg)
                nc.tensor.matmul(st[:, cg, :], mc[:, cc], rawc[:],
                                 start=True, stop=True)
            jb = slice(go, go + gn)
            tnh = pch.tile([128, gn], f32, name=f"tnh{b}_{go}", tag="tnh")
            nc.scalar.activation(tnh[:], st[:, :, 64:65].squeeze(2),
                                 AF.Tanh, bias=wt["bih"][:], scale=0.5)
            gate = pch.tile([128, gn], f32, name=f"gt{b}_{go}", tag="gate")
            nc.vector.tensor_scalar(out=gate[:], in0=tnh[:], scalar1=1.0,
                                    scalar2=0.5, op0=OP.add, op1=OP.mult)
            nc.vector.tensor_tensor(
                rcg[:, jb, 0:64], st[:, :, 0:64],
                gate[:].unsqueeze(2).broadcast_to([128, gn, 64]), OP.mult)
            nc.vector.tensor_tensor(
                rcg[:, jb, 64:67], cdl[:, jb, :],
                st[:, :, 65:66].broadcast_to([128, gn, 3]), OP.mult)

        def scatter_part(b, j0, j1):
            stt = state[b]
            if j0 == 0:
                stt["aggT"] = pagg.tile([67, 128], f32,
                                        name=f"aggT{b}", tag="aggT")
            aggT, rcg, ohts = stt["aggT"], stt["rcg"], stt["ohts"]
            for j in range(j0, j1):
                nc.tensor.matmul(aggT[:, :], rcg[:, j, :],
                                 ohts[:, j, :],
                                 start=(j == 0), stop=(j == NCH - 1))
            if j1 == NCH:
                nc.vector.tensor_copy(aggm[:, b * 128:(b + 1) * 128],
                                      aggT[0:64, :])
                nc.vector.tensor_copy(aggc[:, b * 128:(b + 1) * 128],
                                      aggT[64:67, :])
                state.pop(b)

        def back_steps(b):
            steps = [lambda gi=gi: flip_group(b, gi) for gi in range(len(GRPS))]
            steps.append(lambda: scatter_part(b, 0, NCH // 2))
            steps.append(lambda: scatter_part(b, NCH // 2, NCH))
            return steps

        def front_steps(b):
            steps = [lambda s=s: front_s1(b, s) for s in range(NSUP)]
            if STAGE >= 3:
                steps += [lambda s=s: front_s23(b, s) for s in range(NSUP)]
            return steps

        def front(b):
            front_dma(b)
            if STAGE >= 3:
                for f in front_steps(b):
                    f()

        def back(b):
            for s in back_steps(b):
                s()

        for b in range(NBLK):
            front(b)
            if STAGE >= 4 and b > 0:
                back(b - 1)
            if STAGE >= 5 and b > 1:
                phase_c(b - 2)
        if STAGE >= 4:
            back(NBLK - 1)
        if STAGE >= 5:
            phase_c(NBLK - 2)
            phase_c(NBLK - 1)


def kernel(**inputs):
    ei = np.asarray(inputs["edge_indices"])
    start = ei[0].astype(np.int64)
    end = ei[1].astype(np.int64)
    ef = _f(inputs["edge_features"])
    nfi = _f(inputs["node_features_input"])
    coords = nfi[:, 0:3]
    cd_all = coords[start] - coords[end]
    cdn_all = np.sqrt((cd_all ** 2).sum(1)).astype(np.float32)
    deg = np.bincount(start, minlength=N).astype(np.float32)
    invcnt_all = (1.0 / np.maximum(deg, 1.0)).astype(np.float32)
    nf_bf = nfi[:, 6:70].astype(mybir.dt.np(bf16))

    # chunk count from data: ceil(max block load / 128), rounded up to even
    mx = 0
    for c in range(NCORES):
        lo, hi = c * NPC, (c + 1) * NPC
        s = start[(start >= lo) & (start < hi)] - lo
        cnt = np.bincount(s >> 7, minlength=NBLK)
        mx = max(mx, int(cnt.max()))
    NCH = -(-mx // 128)
    NCH += NCH % 2
    NCH = max(NCH, 8)

    w = _prep_weights(inputs)

    in_maps = []
    for c in range(NCORES):
        d = _prep_core(c, NCH, start, end, ef, nfi, nf_bf, cd_all, cdn_all,
                       invcnt_all)
        d.update(w)
        in_maps.append(d)

    if _cache.get("NCH") != NCH:
        _cache["NCH"] = NCH
        _cache["nc"] = _build_program(NCH)
    nc = _cache["nc"]
    _cache["in_maps"] = in_maps
    res = run_bass_kernel_spmd(nc, in_maps, list(range(NCORES)))
    out = np.empty((N, 70), np.float32)
    for c in range(NCORES):
        out[c * NPC:(c + 1) * NPC, 0:6] = res.results[c]["outc"][0:NPC]
        out[c * NPC:(c + 1) * NPC, 6:70] = res.results[c]["outT"][:, 0:NPC].T
    return out


# revision 24
# speedup vs baseline: 1.8183x; 1.0533x over previous
"""EquivariantGraphConvolution (EGNN layer) on 8 Trainium2 NeuronCores.

Strategy (v3.2)
---------------
Nodes are range-partitioned across the 8 cores (6250 nodes each); every edge is
owned by the core that owns its *start* node, so the per-start segment sums are
core-local and no collective is needed.  Per core, edges are bucketed by
128-node start block and padded to a static layout of 49 blocks x NCH chunks x
128 edges, NCH chosen from the data (ceil(max block edges / 128), even).

Key design points (see earlier versions for the v2 gather design):
  * No dma_gather anywhere: the host materializes [nf[start] ; nf[end]] per
    edge slot as one [128, NCH*128] bf16 stream per block (128B/edge).
    Stage 1 is 2 col-tiled matmuls with the [W1a; W1b] stationary plus one
    for the ef/coord-norm terms.
  * The scatter one-hot is built on-chip with one DVE is_equal per block.
  * Segment-sum: rcg [128 edges, 67] bf16 is the matmul stationary, one-hot
    moving, accumulating aggT [67 feats, 128 nodes] over NCH chunks.
  * Activation-table discipline: only Silu/Tanh/Identity (one table set).
    Gate = 0.5*(1+tanh(0.5 logit + 0.5 b_i)).
  * 1/deg is folded into the coord stream, b_n2 into a DVE add; the nf
    output is written feature-major so no transposes are needed in the
    node-update phase; velocity scales go through a DRAM round-trip to
    avoid 49 PE transposes.
"""
import sys
sys.path.insert(0, "/opt/trn_rl_repo")
import contextlib
import os
import numpy as np

import concourse.bass as bass
import concourse.bacc as bacc
import concourse.mybir as mybir
import concourse.tile as tile
from concourse.bass_utils import run_bass_kernel_spmd

f32 = mybir.dt.float32
bf16 = mybir.dt.bfloat16
AF = mybir.ActivationFunctionType
OP = mybir.AluOpType

# ---- problem constants (hardcoded per contract) ----
N = 50000
E = 1_000_000
H = 64
EF = 16
NCORES = 8
NPC = N // NCORES          # 6250 nodes per core
NBLK = 49                  # 128-node blocks per core (49*128 = 6272 >= 6250)
NPAD = NBLK * 128          # 6272
GRP = 6                    # max chunks per flip/gate group

# CoreSim does not implement Silu; substitute Tanh for structural sim checks.
SIM_ACT = os.environ.get("EGNN_SIMACT", "0") == "1"
AF_SILU = AF.Tanh if SIM_ACT else AF.Silu

_cache = {}


def _sup_widths(half):
    """Supertile widths covering `half` columns (each <= 512, mult of 128)."""
    w = [512] * (half // 512)
    if half % 512:
        w.append(half % 512)
    return w


def _f(x):
    return np.ascontiguousarray(x, np.float32)


def _prep_weights(inp):
    """Small weight/constant tensors, identical on all cores."""
    W_e1 = _f(inp["W_e1"])           # [145, 64]
    w = {}
    w["wse"] = W_e1[0:128].copy()                    # [128, 64] start|end rows
    wef = np.zeros((34, 128), np.float32)
    wef[0:16, 0:64] = W_e1[129:145]
    wef[16:17, 0:64] = W_e1[128:129]
    wef[17:33, 64:128] = W_e1[129:145]
    wef[33:34, 64:128] = W_e1[128:129]
    w["wefcdn"] = wef
    W_e2 = _f(inp["W_e2"]); W_c1 = _f(inp["W_c1"])
    bd = np.zeros((128, 128), np.float32)
    bd[0:64, 0:64] = W_e2; bd[64:128, 64:128] = W_e2
    w["wde2"] = bd
    w["wc1d"] = np.concatenate([W_c1, W_c1], axis=0)   # [128, 64], both halves
    W_i = _f(inp["W_i"]); W_c2 = _f(inp["W_c2"])
    # mcA = [msgA ; chA]  -> out cols: 0:64 msg, 64 gate logit, 65 coord weight
    ra = np.zeros((128, 66), np.float32)
    ra[0:64, 0:64] = np.eye(64); ra[0:64, 64:65] = W_i; ra[64:128, 65:66] = W_c2
    w["rawcA"] = ra
    # mcB = [chB ; msgB]
    rb = np.zeros((128, 66), np.float32)
    rb[64:128, 0:64] = np.eye(64); rb[64:128, 64:65] = W_i; rb[0:64, 65:66] = W_c2
    w["rawcB"] = rb
    w["wn1a"] = _f(inp["W_n1"])[0:64].copy()          # [64, 64] nf rows
    w["wn1b"] = _f(inp["W_n1"])[64:128].copy()        # [64, 64] agg rows
    w["wn2"] = _f(inp["W_n2"])
    w["wv1"] = _f(inp["W_v1"])
    w["wv2"] = _f(inp["W_v2"])
    w["eye3"] = np.eye(3, dtype=np.float32)
    w["arange128"] = np.tile(np.arange(128, dtype=np.float32), (128, 1))

    b_e1 = _f(inp["b_e1"]); b_e2 = _f(inp["b_e2"]); b_c1 = _f(inp["b_c1"])
    w["be1s"] = np.concatenate([b_e1, b_e1]).reshape(128, 1)
    w["be2s"] = np.concatenate([b_e2, b_e2]).reshape(128, 1)
    w["bc1s"] = np.concatenate([b_c1, b_c1]).reshape(128, 1)
    w["bih"] = np.full((128, 1), 0.5 * float(np.asarray(inp["b_i"]).ravel()[0]), np.float32)
    w["bn1c"] = _f(inp["b_n1"]).reshape(64, 1)
    w["bn2c"] = _f(inp["b_n2"]).reshape(64, 1)
    w["bv1c"] = _f(inp["b_v1"]).reshape(64, 1)
    w["bv2c"] = np.full((1, 1), float(np.asarray(inp["b_v2"]).ravel()[0]), np.float32)
    return w


def _prep_core(c, NCH, start, end, ef, nfi, nf_bf, cd_all, cdn_all, invcnt_all):
    bfdt = mybir.dt.np(bf16)
    BLKE = NCH * 128
    ES = NBLK * BLKE
    HALF = BLKE // 2
    lo, hi = c * NPC, (c + 1) * NPC
    sel = (start >= lo) & (start < hi)
    eo = np.nonzero(sel)[0]
    s_loc = (start[eo] - lo).astype(np.int64)
    blk = s_loc >> 7
    order = np.argsort(blk, kind="stable")
    eo = eo[order]; s_loc = s_loc[order]; blk = blk[order]
    counts = np.bincount(blk, minlength=NBLK)
    if counts.max() > BLKE:
        raise RuntimeError(f"block overflow: {counts.max()} > {BLKE}")
    starts_ = np.zeros(NBLK, np.int64)
    starts_[1:] = np.cumsum(counts)[:-1]
    within = np.arange(len(eo)) - starts_[blk]
    slots = blk * BLKE + within

    sg = np.full(ES, lo, np.int64)       # global start per slot (pad -> node lo)
    eg = np.zeros(ES, np.int64)          # global end per slot (pad -> node 0)
    lid = np.full(ES, -1.0, np.float32)
    cds = np.zeros((ES, 3), np.float32)
    cdns = np.zeros(ES, np.float32)
    efs = np.zeros((ES, EF), np.float32)
    sg[slots] = start[eo]
    eg[slots] = end[eo]
    lid[slots] = (s_loc & 127).astype(np.float32)
    cds[slots] = cd_all[eo] * invcnt_all[start[eo]][:, None]
    cdns[slots] = cdn_all[eo]
    efs[slots] = ef[eo]

    d = {}
    # [NBLK, 128, BLKE] bf16: rows 0:64 = nf[start].T, 64:128 = nf[end].T
    nfse = np.empty((ES, 128), bfdt)
    nfse[:, 0:64] = nf_bf[sg]
    nfse[:, 64:128] = nf_bf[eg]
    d["nfse"] = np.ascontiguousarray(
        nfse.reshape(NBLK, BLKE, 128).transpose(0, 2, 1))
    # scatter-orientation lid stream [NBLK, 128, NCH] (edge-in-chunk, chunk)
    lidc = lid.reshape(NBLK, NCH, 128)
    d["lidc"] = np.ascontiguousarray(lidc.transpose(0, 2, 1)).astype(bfdt)
    d["cdem"] = cds.reshape(NBLK, NCH, 128, 3).transpose(0, 2, 1, 3).copy()
    # ef|cdn feature-major: rows 0:17 = A-half edge, 17:34 = B-half edge
    efcdn = np.empty((NBLK, 2, HALF, EF + 1), np.float32)
    efcdn[:, :, :, 0:EF] = efs.reshape(NBLK, 2, HALF, EF)
    efcdn[:, :, :, EF] = cdns.reshape(NBLK, 2, HALF)
    d["efcdn"] = np.ascontiguousarray(
        efcdn.transpose(0, 1, 3, 2).reshape(NBLK, 34, HALF)).astype(bfdt)

    nmc = np.zeros((NPAD, 6), np.float32)
    nmc[0:NPC] = nfi[lo:hi, 0:6]
    d["nodec"] = nmc.reshape(NBLK, 128, 6).transpose(1, 0, 2).reshape(128, NBLK * 6).copy()
    nl = np.zeros((64, NPAD), np.float32)
    nl[:, 0:NPC] = nfi[lo:hi, 6:70].T
    d["nfT_local"] = nl
    return d


def _build_program(NCH):
    STAGE = int(os.environ.get("EGNN_STAGE", "5"))
    BLKE = NCH * 128
    HALF = BLKE // 2
    SUPW = _sup_widths(HALF)        # supertile widths
    SUPO = [sum(SUPW[:i]) for i in range(len(SUPW))]
    # flip/gate groups: chunks [o, o+n) per group
    GRPS = []
    o = 0
    while o < NCH:
        n = min(GRP, NCH - o)
        GRPS.append((o, n))
        o += n

    nc = bacc.Bacc("TRN2", target_bir_lowering=False, debug=False,
                   enable_asserts=False, num_devices=NCORES)

    def din(name, shape, dt=f32):
        return nc.dram_tensor(name, list(shape), dt, kind="ExternalInput").ap()

    nfse_d = din("nfse", [NBLK, 128, BLKE], bf16)
    lidc_d = din("lidc", [NBLK, 128, NCH], bf16)
    cdem_d = din("cdem", [NBLK, 128, NCH, 3])
    efcdn_d = din("efcdn", [NBLK, 34, HALF], bf16)
    nodec_d = din("nodec", [128, NBLK * 6])
    nfT_loc_d = din("nfT_local", [64, NPAD])
    wnames = ["wse", "wefcdn", "wde2", "wc1d", "rawcA", "rawcB",
              "wn1a", "wn1b", "wn2", "wv1", "wv2", "eye3", "arange128",
              "be1s", "be2s", "bc1s", "bih", "bn1c", "bn2c", "bv1c", "bv2c"]
    wshapes = {"wse": [128, 64], "wefcdn": [34, 128],
               "wde2": [128, 128], "wc1d": [128, 64],
               "rawcA": [128, 66], "rawcB": [128, 66],
               "wn1a": [64, 64], "wn1b": [64, 64], "wn2": [64, 64],
               "wv1": [64, 64], "wv2": [64, 1], "eye3": [3, 3],
               "arange128": [128, 128],
               "be1s": [128, 1], "be2s": [128, 1], "bc1s": [128, 1],
               "bih": [128, 1], "bn1c": [64, 1], "bn2c": [64, 1],
               "bv1c": [64, 1], "bv2c": [1, 1]}
    wd = {n: din(n, wshapes[n]) for n in wnames}
    outc_d = nc.dram_tensor("outc", [NPAD, 6], f32, kind="ExternalOutput").ap()
    outT_d = nc.dram_tensor("outT", [64, NPAD], f32, kind="ExternalOutput").ap()
    vs_dram = nc.dram_tensor("vs_dram", [NPAD], f32).ap()

    # weights that are matmul operands in the bf16 pipeline
    BF_W = ("wse", "wefcdn", "wde2", "wc1d", "rawcA", "rawcB", "arange128")

    with tile.TileContext(nc) as tc, contextlib.ExitStack() as ctx:
        wpool = ctx.enter_context(tc.tile_pool(name="w", bufs=1))
        wt = {}
        for n in wnames:
            dt = bf16 if n in BF_W else f32
            t = wpool.tile(wshapes[n], dt, name=f"wt_{n}")
            if dt == f32:
                nc.sync.dma_start(t[:], wd[n][:])
            else:
                tf = wpool.tile(wshapes[n], f32, name=f"wtf_{n}")
                nc.sync.dma_start(tf[:], wd[n][:])
                nc.vector.tensor_copy(t[:], tf[:])
            wt[n] = t
        nodec = wpool.tile([128, NBLK * 6], f32, name="nodec")
        nc.sync.dma_start(nodec[:], nodec_d[:])
        nfT_loc = wpool.tile([64, NPAD], f32, name="nfT_loc")
        nc.sync.dma_start(nfT_loc[:], nfT_loc_d[:])
        vscale = wpool.tile([128, NBLK], f32, name="vscale")
        aggm = wpool.tile([64, NPAD], f32, name="aggm")
        aggc = wpool.tile([3, NPAD], f32, name="aggc")

        # ---------- Phase B: velocity MLP -> vscale [128, NBLK] ----------
        with tc.tile_pool(name="pb", bufs=2) as pb, \
             tc.tile_pool(name="pbp", bufs=2, space="PSUM") as pbp:
            tiles = [(j * 512, 512) for j in range(NPAD // 512)]
            if NPAD % 512:
                tiles.append((NPAD // 512 * 512, NPAD % 512))
            for (o, L) in tiles:
                vps = pbp.tile([64, L], f32, name=f"vps{o}", tag="vps")
                nc.tensor.matmul(vps[:], wt["wv1"][:], nfT_loc[:, o:o + L])
                vh = pb.tile([64, L], f32, name=f"vh{o}", tag="vh")
                nc.scalar.activation(vh[:], vps[:], AF_SILU, bias=wt["bv1c"][:])
                sps = pbp.tile([1, L], f32, name=f"sps{o}", tag="sps")
                nc.tensor.matmul(sps[:], wt["wv2"][:], vh[:])
                vsc = pb.tile([1, L], f32, name=f"vsc{o}", tag="vsc")
                nc.scalar.activation(vsc[:], sps[:], AF.Identity, bias=wt["bv2c"][:])
                nc.sync.dma_start(vs_dram[o:o + L].unsqueeze(0), vsc[:])
            # read back node-block-major: vscale[p, b] = vs_dram[b*128 + p]
            nc.sync.dma_start(vscale[:],
                              vs_dram[:].rearrange("(b p) -> p b", p=128))

        # ---------- Edge sweep (node update fused per block) ----------
        if STAGE >= 2:
            _edge_sweep(nc, tc, STAGE, NCH, SUPW, SUPO, GRPS, wt,
                        nfse_d, lidc_d, cdem_d, efcdn_d, nfT_loc,
                        aggm, aggc, nodec, vscale, outc_d, outT_d)

    nc.compile()
    return nc


def _edge_sweep(nc, tc, STAGE, NCH, SUPW, SUPO, GRPS, wt,
                nfse_d, lidc_d, cdem_d, efcdn_d, nfT_loc,
                aggm, aggc, nodec, vscale, outc_d, outT_d):
    BLKE = NCH * 128
    HALF = BLKE // 2
    NHC = NCH // 2
    NSUP = len(SUPW)

    def mc_of_chunk(mcs, j):
        """Map chunk j to (mc tile, rawc, 128-col slice within supertile)."""
        half, jj = (0, j) if j < NHC else (1, j - NHC)
        col = jj * 128
        for s in range(NSUP):
            if col < SUPO[s] + SUPW[s]:
                off = col - SUPO[s]
                rawc = wt["rawcA"] if half == 0 else wt["rawcB"]
                return mcs[s][half], rawc, slice(off, off + 128)
        raise AssertionError

    with tc.tile_pool(name="pg", bufs=3) as pg, \
         tc.tile_pool(name="pcd", bufs=5) as pcd, \
         tc.tile_pool(name="ph", bufs=3) as ph, \
         tc.tile_pool(name="pe", bufs=7) as pe, \
         tc.tile_pool(name="pch", bufs=6) as pch, \
         tc.tile_pool(name="poh", bufs=2) as poh, \
         tc.tile_pool(name="px1", bufs=3, space="PSUM") as px1, \
         tc.tile_pool(name="pmc", bufs=2, space="PSUM") as pmc, \
         tc.tile_pool(name="pst", bufs=2, space="PSUM") as pst, \
         tc.tile_pool(name="pagg", bufs=1, space="PSUM") as pagg:

        def phase_c(b0, nb=1):
            W = nb * 128
            cols = slice(b0 * 128, b0 * 128 + W)
            n1 = pagg.tile([64, W], f32, name=f"n1{b0}", tag="aggT")
            nc.tensor.matmul(n1[:], wt["wn1a"][:], nfT_loc[:, cols],
                             start=True, stop=False)
            nc.tensor.matmul(n1[:], wt["wn1b"][:], aggm[:, cols],
                             start=False, stop=True)
            hn = pch.tile([64, W], f32, name=f"hn{b0}", tag="hn")
            nc.scalar.activation(hn[:], n1[:], AF_SILU, bias=wt["bn1c"][:])
            n2 = pagg.tile([64, W], f32, name=f"n2{b0}", tag="aggT")
            nc.tensor.matmul(n2[:], wt["wn2"][:], hn[:])
            hn2 = pch.tile([64, W], f32, name=f"hn2{b0}", tag="hn2")
            nc.vector.tensor_tensor(
                hn2[:], n2[:], wt["bn2c"][:].broadcast_to([64, W]), OP.add)
            ot67 = pch.tile([64, W], f32, name=f"ot67{b0}", tag="ot67")
            nc.vector.tensor_tensor(ot67[:], nfT_loc[:, cols], hn2[:], OP.add)
            nc.sync.dma_start(outT_d[:, cols], ot67[:])
            # coords/vels (node-major)
            nmb = nodec[:, b0 * 6:(b0 + nb) * 6].rearrange(
                "p (t c) -> p t c", t=nb)
            ctp = pagg.tile([128, nb, 3], f32, name=f"ctp{b0}", tag="aggT")
            for t in range(nb):
                nc.tensor.transpose(ctp[:, t, :],
                                    aggc[:, (b0 + t) * 128:(b0 + t + 1) * 128],
                                    wt["eye3"][:])
            otc = pch.tile([128, nb, 6], f32, name=f"otc{b0}", tag="otc")
            t2 = pch.tile([128, nb, 3], f32, name=f"t2{b0}", tag="t2")
            nc.vector.tensor_tensor(
                t2[:], nmb[:, :, 3:6],
                vscale[:, b0:b0 + nb].unsqueeze(2).broadcast_to([128, nb, 3]),
                OP.mult)
            nc.vector.tensor_copy(otc[:, :, 3:6], nmb[:, :, 3:6])
            t3 = pch.tile([128, nb, 3], f32, name=f"t3{b0}", tag="t3")
            nc.vector.tensor_tensor(t3[:], ctp[:], t2[:], OP.add)
            nc.vector.tensor_tensor(otc[:, :, 0:3], t3[:], nmb[:, :, 0:3], OP.add)
            nc.sync.dma_start(
                outc_d[b0 * 128:(b0 + nb) * 128, :].rearrange(
                    "(t p) c -> p t c", t=nb),
                otc[:])

        state = {}

        def front_dma(b):
            nfse = pg.tile([128, BLKE], bf16, name=f"nfse{b}", tag="nfse")
            nc.sync.dma_start(nfse[:], nfse_d[b])
            lidt = pg.tile([128, NCH], bf16, name=f"lidt{b}", tag="lidt")
            nc.sync.dma_start(lidt[:], lidc_d[b])
            cdl = pcd.tile([128, NCH, 3], f32, name=f"cdl{b}", tag="cdl")
            nc.sync.dma_start(cdl[:], cdem_d[b])
            eftb = pg.tile([34, HALF], bf16, name=f"eftb{b}", tag="eftb")
            nc.sync.dma_start(eftb[:], efcdn_d[b])

            # scatter one-hot [128 edge-in-chunk, NCH chunk, 128 node] on-chip
            ohts = poh.tile([128, NCH, 128], bf16, name=f"ohts{b}", tag="ohts")
            nc.vector.tensor_tensor(
                ohts[:],
                lidt[:].unsqueeze(2).broadcast_to([128, NCH, 128]),
                wt["arange128"][:].unsqueeze(1).broadcast_to([128, NCH, 128]),
                OP.is_equal)
            state[b] = {"nfse": nfse, "cdl": cdl, "eftb": eftb, "ohts": ohts,
                        "h1s": [], "mcs": []}
            if STAGE == 2:
                nc.any.tensor_copy(aggm[:, b * 128:(b + 1) * 128],
                                   ohts[0:64, 0, :])

        def front_s1(b, s):
            stt = state[b]
            w = SUPW[s]
            sl = slice(SUPO[s], SUPO[s] + w)
            slh = slice(HALF + SUPO[s], HALF + SUPO[s] + w)
            x1 = px1.tile([128, w], f32, name=f"x1{b}_{s}", tag="x1")
            nc.tensor.matmul(x1[0:64, :], wt["wse"][:], stt["nfse"][:, sl],
                             start=True, stop=False, skip_group_check=True)
            nc.tensor.matmul(x1[64:128, :], wt["wse"][:], stt["nfse"][:, slh],
                             start=True, stop=False,
                             tile_position=(0, 64), skip_group_check=True)
            nc.tensor.matmul(x1[:], wt["wefcdn"][:], stt["eftb"][:, sl],
                             start=False, stop=True, skip_group_check=True)
            h1 = ph.tile([128, w], bf16, name=f"h1{b}_{s}", tag="h1")
            nc.scalar.activation(h1[:], x1[:], AF_SILU, bias=wt["be1s"][:])
            stt["h1s"].append(h1)

        def front_s23(b, s):
            stt = state[b]
            w = SUPW[s]
            mp = pmc.tile([128, w], f32, name=f"mp{b}_{s}", tag="mmid")
            nc.tensor.matmul(mp[:], wt["wde2"][:], stt["h1s"][s][:])
            mcA = pe.tile([128, w], bf16, name=f"mcA{b}_{s}", tag="mcA")
            mcB = pe.tile([128, w], bf16, name=f"mcB{b}_{s}", tag="mcB")
            nc.scalar.activation(mcA[0:64, :], mp[0:64, :], AF_SILU,
                                 bias=wt["be2s"][0:64, :])
            nc.scalar.activation(mcB[64:128, :], mp[64:128, :], AF_SILU,
                                 bias=wt["be2s"][64:128, :])
            cpx = pmc.tile([128, w], f32, name=f"cp{b}_{s}", tag="mmid")
            nc.tensor.matmul(cpx[64:128, :], wt["wc1d"][0:64, :], mcA[0:64, :],
                             start=True, stop=True,
                             tile_position=(0, 64), skip_group_check=True)
            nc.tensor.matmul(cpx[0:64, :], wt["wc1d"][64:128, :], mcB[64:128, :],
                             start=True, stop=True,
                             tile_position=(64, 0), skip_group_check=True)
            nc.scalar.activation(mcA[64:128, :], cpx[64:128, :], AF_SILU,
                                 bias=wt["bc1s"][64:128, :])
            nc.scalar.activation(mcB[0:64, :], cpx[0:64, :], AF_SILU,
                                 bias=wt["bc1s"][0:64, :])
            stt["mcs"].append((mcA, mcB))
            if STAGE == 3 and s == 0:
                nc.any.tensor_copy(aggm[:, b * 128:(b + 1) * 128],
                                   mcA[0:64, 0:128])

        def flip_group(b, gi):
            stt = state[b]
            if gi == 0:
                stt["rcg"] = pch.tile([128, NCH, 67], bf16,
                                      name=f"rcg{b}", tag="rcg")
            rcg, cdl = stt["rcg"], stt["cdl"]
            (go, gn) = GRPS[gi]
            st = pst.tile([128, gn, 66], f32, name=f"st{b}_{go}", tag="st")
            for cg in range(gn):
                mc, rawc, cc = mc_of_chunk(stt["mcs"], go + cg)
                nc.tensor.matmul(st[:, cg, :], mc[:, cc], rawc[:],
                                 start=True, stop=True)
            jb = slice(go, go + gn)
            tnh = pch.tile([128, gn], f32, name=f"tnh{b}_{go}", tag="tnh")
            nc.scalar.activation(tnh[:], st[:, :, 64:65].squeeze(2),
                                 AF.Tanh, bias=wt["bih"][:], scale=0.5)
            gate = pch.tile([128, gn], f32, name=f"gt{b}_{go}", tag="gate")
            nc.vector.tensor_scalar(out=gate[:], in0=tnh[:], scalar1=1.0,
                                    scalar2=0.5, op0=OP.add, op1=OP.mult)
            nc.vector.tensor_tensor(
                rcg[:, jb, 0:64], st[:, :, 0:64],
                gate[:].unsqueeze(2).broadcast_to([128, gn, 64]), OP.mult)
            nc.vector.tensor_tensor(
                rcg[:, jb, 64:67], cdl[:, jb, :],
                st[:, :, 65:66].broadcast_to([128, gn, 3]), OP.mult)

        def scatter_part(b, j0, j1):
            stt = state[b]
            if j0 == 0:
                stt["aggT"] = pagg.tile([67, 128], f32,
                                        name=f"aggT{b}", tag="aggT")
            aggT, rcg, ohts = stt["aggT"], stt["rcg"], stt["ohts"]
            for j in range(j0, j1):
                nc.tensor.matmul(aggT[:, :], rcg[:, j, :],
                                 ohts[:, j, :],
                                 start=(j == 0), stop=(j == NCH - 1))
            if j1 == NCH:
                nc.scalar.activation(aggm[:, b * 128:(b + 1) * 128],
                                     aggT[0:64, :], AF.Identity)
                nc.scalar.activation(aggc[:, b * 128:(b + 1) * 128],
                                     aggT[64:67, :], AF.Identity)
                state.pop(b)

        def back_steps(b):
            steps = [lambda gi=gi: flip_group(b, gi) for gi in range(len(GRPS))]
            steps.append(lambda: scatter_part(b, 0, NCH // 2))
            steps.append(lambda: scatter_part(b, NCH // 2, NCH))
            return steps

        def front_steps(b):
            steps = [lambda s=s: front_s1(b, s) for s in range(NSUP)]
            if STAGE >= 3:
                steps += [lambda s=s: front_s23(b, s) for s in range(NSUP)]
            return steps

        def front(b):
            front_dma(b)
            if STAGE >= 3:
                for f in front_steps(b):
                    f()

        def back(b):
            for s in back_steps(b):
                s()

        for b in range(NBLK):
            front(b)
            if STAGE >= 4 and b > 0:
                back(b - 1)
            if STAGE >= 5 and b >= 3 and b % 2 == 1:
                phase_c(b - 3, 2)
        if STAGE >= 4:
            back(NBLK - 1)
        if STAGE >= 5:
            done = ((NBLK - 2) // 2) * 2    # phase_c emitted for [0, done)
            b = done
            while b < NBLK:
                nb = min(2, NBLK - b)
                phase_c(b, nb)
                b += nb


def kernel(**inputs):
    ei = np.asarray(inputs["edge_indices"])
    start = ei[0].astype(np.int64)
    end = ei[1].astype(np.int64)
    ef = _f(inputs["edge_features"])
    nfi = _f(inputs["node_features_input"])
    coords = nfi[:, 0:3]
    cd_all = coords[start] - coords[end]
    cdn_all = np.sqrt((cd_all ** 2).sum(1)).astype(np.float32)
    deg = np.bincount(start, minlength=N).astype(np.float32)
    invcnt_all = (1.0 / np.maximum(deg, 1.0)).astype(np.float32)
    nf_bf = nfi[:, 6:70].astype(mybir.dt.np(bf16))

    # chunk count from data: ceil(max block load / 128), rounded up to even
    mx = 0
    for c in range(NCORES):
        lo, hi = c * NPC, (c + 1) * NPC
        s = start[(start >= lo) & (start < hi)] - lo
        cnt = np.bincount(s >> 7, minlength=NBLK)
        mx = max(mx, int(cnt.max()))
    NCH = -(-mx // 128)
    NCH += NCH % 2
    NCH = max(NCH, 8)

    w = _prep_weights(inputs)

    in_maps = []
    for c in range(NCORES):
        d = _prep_core(c, NCH, start, end, ef, nfi, nf_bf, cd_all, cdn_all,
                       invcnt_all)
        d.update(w)
        in_maps.append(d)

    if _cache.get("NCH") != NCH:
        _cache["NCH"] = NCH
        _cache["nc"] = _build_program(NCH)
    nc = _cache["nc"]
    _cache["in_maps"] = in_maps
    res = run_bass_kernel_spmd(nc, in_maps, list(range(NCORES)))
    out = np.empty((N, 70), np.float32)
    for c in range(NCORES):
        out[c * NPC:(c + 1) * NPC, 0:6] = res.results[c]["outc"][0:NPC]
        out[c * NPC:(c + 1) * NPC, 6:70] = res.results[c]["outT"][:, 0:NPC].T
    return out
